# revision 8
# baseline (speedup 1.0000x reference)
"""Trainium2 Bass kernel for nn_CustomModel_88862873354402 (gnn_message_passing).

Model (per batch b of 32, N=65536 nodes, D=16 features):
    h        = relu(X @ mw1 + mb1)               [N, 64]
    messages = h @ mw2 + mb2                     [N, 32]
    msg_sum  = sum_n messages                    [32]      (broadcast to all nodes)
    feat     = [msg_sum, x_last]                 [N, 33]
    g        = relu(feat @ iw1 + ib1)            [N, 64]
    out      = g @ iw2 + ib2                     [N, 3]

Algebraic structure exploited:
 1. msg_sum needs only sum_n relu(X @ mw1 + mb1), never per-node messages.
 2. Stage 2 collapses to an exact per-batch affine map out = A_b*x_last + B_b
    because |c_h| >> |w_h*x|; straddling hinges (classified on device, verified
    host-side in fp64 with a safety margin) are patched on host (n=0 here).

v7 design (117 us -> target ~100 us):
 - Single merged launch; X packed host-side into bf16 feature-major quadrant
   layout; four 32-row quadrant matmuls per 2048-col round at distinct
   tile_position row groups (bf16: 1 cyc/col).
 - relu+sum drains IN-PLACE on PSUM (out == in tile): ACT takes psum bank
   pair 0 (Relu bias=mb1, accum_out), DVE bank pair 1 (max(z,-b) + accum);
   in-place avoids the SBUF-output access penalty on ACT (222 vs 172 cyc)
   and drops the trash tiles.
 - Affine apply is EXACT (the model's output is affine in x_last given the
   hinge mask): y_o = A_o*x + B_o via tensor_scalar(mult,add) on the
   otherwise-idle GpSimd(Pool) engine - off the ACT/DVE critical path; the
   v6 Relu sign-trick and host sign-restore are gone. Last batch splits
   planes DVE/Pool/Pool so the tail applies run in parallel.
 - Per-batch coefficient chain on device as in v6 (generator interleaved
   into the NEXT batch's rounds; plain ops on GpSimd, reductions/max on
   DVE, 2 tiny PE matmuls with all-ones stationary for cross-partition
   reduce+broadcast). Sign-trick steps removed (3 fewer serial ops).
 - _prune_drain_deps now also prunes the PE queue's waits (in-order
   retirement makes repeated same-producer waits redundant).
 - DMA order: w1big + biasx + first x chunk first; other consts (needed
   only by the chain, ~25us in) issued after, so round 0 starts sooner.
"""
import sys

if "/opt/trn_rl_repo" not in sys.path:
    sys.path.insert(0, "/opt/trn_rl_repo")

from contextlib import ExitStack

import ml_dtypes
import numpy as np

import bass_rust as _bass_rust
import concourse.bass as bass
import concourse.tile as tile
from concourse import mybir
from concourse.bass_utils import run_bass_kernel_spmd

F32 = mybir.dt.float32
BF16 = mybir.dt.bfloat16
AF = mybir.ActivationFunctionType
ALU = mybir.AluOpType
BFNP = ml_dtypes.bfloat16

B, N, D = 32, 65536, 16
H, M, OUT = 64, 32, 3
NCORES = 8
BL = B // NCORES            # batches per core
CHUNK = 16384               # nodes per chunk tile
NCH = N // CHUNK            # chunks per batch
QCOLS = 2048                # moving cols per quadrant per chunk (2 nodes/col)
RND = 4                     # rounds per chunk (512 cols per quadrant each)
NRND = NCH * RND            # rounds per batch
NJ = N // 128               # apply free dim per batch
HCOLS = QCOLS // 2          # per-round cols drained by each of ACT / DVE

LAST_EXEC_NS = []

_cache = {}


def _finalize(nc):
    # Legalize for walrus: at most one sync wait per instruction.
    _bass_rust.move_matmul_waits_to_ldweights(nc.m)
    _bass_rust.generate_event_semaphores(nc)


_COMPUTE_ENGINES = ("EngineType.PE", "EngineType.Activation", "EngineType.DVE",
                    "EngineType.Pool", "EngineType.GpSimd", "EngineType.SP")


def _prune_drain_deps(nc):
    """Reduce sync deps using in-order engine-queue guarantees.

    Each compute engine retires its instructions in FIFO order, so:
    - several sync-deps on the same producer engine collapse to the latest;
    - a dep on instruction T is droppable if an earlier instruction on the
      SAME consumer engine already kept a sync-dep on T (any target kind,
      including a specific DMA instruction);
    - same-engine deps are implicit.
    Fewer deps -> fewer InstEventSemaphore instructions on engine queues.
    """
    f = list(nc.m.functions)[0]
    for blk in f.blocks:
        insts = list(blk.instructions)
        by_name = {i.name: i for i in insts}
        order = {i.name: k for k, i in enumerate(insts)}
        kept = set()  # (consumer_engine, dep_target) pairs already waited on
        for i in insts:
            eng = str(i.engine)
            if eng not in _COMPUTE_ENGINES:
                continue
            deps = [d for d, info in i.dependency_edges()]
            by_prod = {}
            for d in deps:
                if d not in by_name:
                    continue
                peng = str(by_name[d].engine)
                if peng == eng:
                    i.remove_dependency(d)
                elif (eng, d) in kept:
                    i.remove_dependency(d)
                elif peng in _COMPUTE_ENGINES and peng != "EngineType.SP":
                    by_prod.setdefault(peng, []).append(d)
                else:
                    kept.add((eng, d))
            for peng, ds in by_prod.items():
                ds.sort(key=lambda d: order[d])
                for d in ds[:-1]:
                    i.remove_dependency(d)
                kept.add((eng, ds[-1]))


def _build_merged():
    """Single launch: stage-1 relu-sum rounds + on-device per-batch affine
    coefficients + exact affine apply + y writeback. Host verifies hinge
    classification afterwards (fp64) and patches y for straddling hinges."""
    nc = bass.Bass()
    xq_in = nc.declare_dram_parameter("xq", [BL, NCH, 128, QCOLS], BF16, isOutput=False)
    w1_in = nc.declare_dram_parameter("w1big", [128, 128], BF16, isOutput=False)
    b1_in = nc.declare_dram_parameter("biasx", [128, 2], F32, isOutput=False)
    c128_in = nc.declare_dram_parameter("cst128", [128, 71], F32, isOutput=False)
    c64_in = nc.declare_dram_parameter("cst64", [64, 40], F32, isOutput=False)
    wx_in = nc.declare_dram_parameter("wx", [64, BL * 4], F32, isOutput=False)
    on_in = nc.declare_dram_parameter("ones1", [128, 128], F32, isOutput=False)
    xl_in = nc.declare_dram_parameter("xl", [128, BL * NJ], F32, isOutput=False)
    hacc_out = nc.declare_dram_parameter(
        "hacc", [BL, 128, 2 * NRND], F32, isOutput=True
    )
    mask_out = nc.declare_dram_parameter("mask", [BL, 64, 1], F32, isOutput=True)
    y_out = nc.declare_dram_parameter("y", [BL, OUT, N], F32, isOutput=True)

    with tile.TileContext(nc) as tc, ExitStack() as ctx:
        const_pool = ctx.enter_context(tc.tile_pool(name="const", bufs=1))
        xin_pool = ctx.enter_context(tc.tile_pool(name="xin", bufs=4))
        xb_pool = ctx.enter_context(tc.tile_pool(name="xb", bufs=2))
        acc_pool = ctx.enter_context(tc.tile_pool(name="acc", bufs=4))
        ch_pool = ctx.enter_context(tc.tile_pool(name="chain", bufs=2))
        ypool = ctx.enter_context(tc.tile_pool(name="yb", bufs=2))
        psA_pool = ctx.enter_context(tc.tile_pool(name="psA", bufs=2, space="PSUM"))
        psV_pool = ctx.enter_context(tc.tile_pool(name="psV", bufs=2, space="PSUM"))

        # group 1: needed by round 0 (stationary weights + drain bias)
        w1big = const_pool.tile([128, 128], BF16)
        nc.sync.dma_start(out=w1big[:], in_=w1_in[:, :])
        biasx = const_pool.tile([128, 2], F32)
        nc.sync.dma_start(out=biasx[:], in_=b1_in[:, :])
        bias = biasx[:, 0:1]
        negb = biasx[:, 1:2]
        # group 2 tiles (DMAs deferred until after the first x chunk so the
        # first round's data wins the queue)
        cst128 = const_pool.tile([128, 71], F32)
        cst64 = const_pool.tile([64, 40], F32)
        wxall = const_pool.tile([64, BL * 4], F32)
        ones_s = const_pool.tile([128, 128], F32)

        def emit_group2_dmas():
            nc.sync.dma_start(out=cst128[:], in_=c128_in[:, :])
            nc.sync.dma_start(out=cst64[:], in_=c64_in[:, :])
            nc.sync.dma_start(out=wxall[:], in_=wx_in[:, :])
            nc.sync.dma_start(out=ones_s[:], in_=on_in[:, :])

        w2big_s = cst128[:, 0:32]     # w2big[h or h+64, m] = mw2[h, m]
        ib2rep = cst128[:, 64:70]     # cols 3:6 = ib2, cols 0:3 = 0
        bcorr32 = cst128[:, 70:71]    # n_dve_elems * [mb1;mb1] / (2*NRND)
        iw1T_s = cst64[:, 0:32]       # iw1T[h, m] = iw1[m, h]
        iw2_s = cst64[:, 32:35]
        iw2w_s = cst64[:, 35:38]      # iw2[h, o] * w[h]
        ib1c32 = cst64[:, 38:39]      # ib1' / 32

        def chain_gen(b, acc, xb_t, last=False):
            # ge: engine for the plain add/mult chain ops. GpSimd for
            # interleaved batches; DVE for the LAST batch, where serial
            # latency (not queue occupancy) matters. Reductions / is_gt /
            # PSUM reads must stay on DVE (Pool rejects them).
            ge = nc.vector if last else nc.gpsimd
            wxb = wxall[:, 4 * b : 4 * b + 4]
            # hsc = sum over all 2*NRND acc cols (+ folded n_dve*mb1 corr)
            hsc = ch_pool.tile([128, 1], F32, tag="hsc")
            tr1 = ch_pool.tile([128, 2 * NRND], F32, tag="tr1")
            nc.vector.tensor_scalar(tr1[:], acc[:], bcorr32, None, op0=ALU.add,
                                    op1=ALU.add, accum_out=hsc[:])
            yield
            t32 = ch_pool.tile([128, 32], F32, tag="t32")
            nc.vector.tensor_scalar(t32[:], w2big_s, hsc[:], None, op0=ALU.mult)
            yield
            msg_ps = psA_pool.tile([64, 32], F32, tag="psa")
            nc.tensor.matmul(msg_ps[:], ones_s[:, 0:64], t32[:],
                             start=True, stop=True)
            yield
            # cc = iw1[:M].T @ msg + ib1' (ib1' folded into the reduce;
            # it carries ib1 + the N*mb2 contribution, host-side)
            tcm = ch_pool.tile([64, 32], F32, tag="tcm")
            if last:
                nc.vector.tensor_tensor(tcm[:], iw1T_s, msg_ps[:], op=ALU.mult)
            else:
                msgc = ch_pool.tile([64, 32], F32, tag="msgc")
                nc.vector.tensor_copy(msgc[:], msg_ps[:])
                yield
                ge.tensor_tensor(tcm[:], iw1T_s, msgc[:], op=ALU.mult)
            cc = ch_pool.tile([64, 1], F32, tag="cc")
            tc0 = ch_pool.tile([64, 32], F32, tag="tc0")
            nc.vector.tensor_scalar(tc0[:], tcm[:], ib1c32, None, op0=ALU.add,
                                    op1=ALU.add, accum_out=cc[:])
            yield
            # hinge mask via host-precomputed threshold tau (band errors are
            # re-verified and patched host-side in fp64)
            on = ch_pool.tile([64, 1], F32, tag="on")
            nc.vector.tensor_tensor(on[:], cc[:], wxb[:, 0:1], op=ALU.is_gt)
            yield
            occ = ch_pool.tile([64, 1], F32, tag="occ")
            ge.tensor_tensor(occ[:], cc[:], on[:], op=ALU.mult)
            t6 = ch_pool.tile([64, 6], F32, tag="t6")
            nc.vector.tensor_scalar(t6[:, 0:3], iw2w_s, on[:], None,
                                    op0=ALU.mult)
            nc.vector.tensor_scalar(t6[:, 3:6], iw2_s, occ[:], None,
                                    op0=ALU.mult)
            yield
            scb_ps = psV_pool.tile([128, 6], F32, tag="psv")
            nc.tensor.matmul(scb_ps[:], ones_s[0:64, :], t6[:],
                             start=True, stop=True)
            yield
            sc2 = ch_pool.tile([128, 6], F32, tag="sc2")
            if last:
                nc.vector.tensor_tensor(sc2[:], scb_ps[:], ib2rep, op=ALU.add)
            else:
                scb = ch_pool.tile([128, 6], F32, tag="scb")
                nc.vector.tensor_copy(scb[:], scb_ps[:])
                yield
                ge.tensor_tensor(sc2[:], scb[:], ib2rep, op=ALU.add)
            yield
            # exact affine apply: y_o = A_o*x + B_o (no Relu needed - the
            # hinge-collapsed model IS affine). Off the ACT/DVE round path:
            # interleaved batches apply on GpSimd; last batch fans planes
            # across DVE/GpSimd so they run in parallel.
            yb = ypool.tile([128, OUT, NJ], F32)
            yv = y_out[b, :, :].rearrange("o (p j) -> p o j", p=128)
            for o in range(OUT):
                eng = nc.vector if (last and o == 0) else nc.gpsimd
                eng.tensor_scalar(
                    yb[:, o, :], xb_t[:],
                    sc2[:, o : o + 1], sc2[:, 3 + o : 4 + o],
                    op0=ALU.mult, op1=ALU.add,
                )
                nc.sync.dma_start(out=yv[:, o : o + 1, :], in_=yb[:, o : o + 1, :])
                if o < OUT - 1 and not last:
                    yield
            nc.sync.dma_start(out=mask_out[b, :, :], in_=on[:])

        pending = None
        for b in range(BL):
            acc = acc_pool.tile([128, 2 * NRND], F32, tag="acc")
            xb_t = xb_pool.tile([128, NJ], F32, tag="xb")
            for c in range(NCH):
                xt = xin_pool.tile([128, QCOLS], BF16)
                if b == 0 and c == 0:
                    # split first chunk so round 0 starts ~4x sooner
                    for r4 in range(RND):
                        nc.sync.dma_start(
                            out=xt[:, 512 * r4 : 512 * (r4 + 1)],
                            in_=xq_in[b, c, :, 512 * r4 : 512 * (r4 + 1)],
                        )
                    emit_group2_dmas()
                else:
                    nc.sync.dma_start(out=xt[:], in_=xq_in[b, c, :, :])
                for r in range(RND):
                    psa = psA_pool.tile([128, HCOLS], F32, tag="psa")
                    psv = psV_pool.tile([128, HCOLS], F32, tag="psv")
                    for q in range(4):
                        ps = psa if q < 2 else psv
                        nc.tensor.matmul(
                            ps[:, 512 * (q % 2) : 512 * (q % 2 + 1)],
                            w1big[32 * q : 32 * (q + 1), :],
                            xt[32 * q : 32 * (q + 1), 512 * r : 512 * (r + 1)],
                            start=True,
                            stop=True,
                            tile_position=(32 * q, 0),
                        )
                    col = c * RND + r
                    # in-place drains: out == in psum tile (no SBUF trash)
                    nc.scalar.activation(
                        psa[:], psa[:], AF.Relu, bias=bias, scale=1.0,
                        accum_out=acc[:, col : col + 1],
                    )
                    nc.vector.tensor_scalar(
                        psv[:], psv[:], negb, None, op0=ALU.max, op1=ALU.add,
                        accum_out=acc[:, NRND + col : NRND + col + 1],
                    )
                    if pending is not None:
                        next(pending, None)
            nc.sync.dma_start(out=xb_t[:], in_=xl_in[:, b * NJ : (b + 1) * NJ])
            nc.sync.dma_start(out=hacc_out[b, :, :], in_=acc[:])
            if pending is not None:
                for _ in pending:
                    pass
            pending = chain_gen(b, acc, xb_t, last=(b == BL - 1))
        for _ in pending:
            pass
    _prune_drain_deps(nc)
    _finalize(nc)
    return nc


def _get_program(key, builder, *args):
    if key not in _cache:
        _cache[key] = builder(*args)
    return _cache[key]


def _pack_x(inputs):
    X = np.ascontiguousarray(np.asarray(inputs, dtype=np.float32))
    Xb = X.astype(BFNP)
    Xq = np.ascontiguousarray(
        Xb.reshape(NCORES, BL, NCH, 4, QCOLS, 2, D).transpose(0, 1, 2, 3, 5, 6, 4)
    ).reshape(NCORES, BL, NCH, 128, QCOLS)
    return X, Xq


def _w1big_biasx(mw1, mb1):
    w1big = np.zeros((128, 128), dtype=np.float32)
    for q in range(4):
        for e in range(2):
            w1big[32 * q + 16 * e : 32 * q + 16 * e + 16,
                  64 * e : 64 * e + 64] = mw1
    w1big = w1big.astype(BFNP)
    biasx = np.zeros((128, 2), dtype=np.float32)
    biasx[:, 0] = np.concatenate([mb1, mb1])
    biasx[:, 1] = -biasx[:, 0]
    return w1big, biasx


def kernel(inputs, mw1, mb1, mw2, mb2, iw1, ib1, iw2, ib2):
    global LAST_EXEC_NS
    LAST_EXEC_NS = []
    X, Xq = _pack_x(inputs)
    mw1 = np.asarray(mw1, dtype=np.float32)
    mb1 = np.asarray(mb1, dtype=np.float32)
    mw2f = np.asarray(mw2, dtype=np.float32)
    mb2f = np.asarray(mb2, dtype=np.float32)
    iw1f = np.asarray(iw1, dtype=np.float32)
    ib1f = np.asarray(ib1, dtype=np.float32)
    iw2f = np.asarray(iw2, dtype=np.float32)
    ib2f = np.asarray(ib2, dtype=np.float32)
    core_ids = list(range(NCORES))
    w1big, biasx = _w1big_biasx(mw1, mb1)
    b1cat = biasx[:, 0].astype(np.float64)
    n_dve_elems = NRND * HCOLS

    xl32 = X[:, :, D - 1]                        # [B, N] fp32
    w = iw1f[M, :].astype(np.float64)            # hinge slopes

    cst128 = np.zeros((128, 71), dtype=np.float32)
    cst128[0:H, 0:32] = mw2f
    cst128[H:128, 0:32] = mw2f
    cst128[:, 32:64] = (np.float64(N) * mb2f.astype(np.float64))[None, :]
    for o in range(OUT):
        cst128[:, 67 + o] = ib2f[o]
    cst128[:, 70] = (n_dve_elems * b1cat / (2 * NRND)).astype(np.float32)
    cst64 = np.zeros((64, 40), dtype=np.float32)
    cst64[:, 0:32] = iw1f[:M].T
    cst64[:, 32:35] = iw2f
    cst64[:, 35:38] = iw2f * iw1f[M, :][:, None]
    ib1p = (
        ib1f.astype(np.float64)
        + iw1f[:M].astype(np.float64).T @ (np.float64(N) * mb2f.astype(np.float64))
    )
    cst64[:, 38] = (ib1p / 32.0).astype(np.float32)

    # per-batch hinge threshold: on = cc > tau,
    # tau = -min(w*xmn, w*xmx) (the 1e-5 fuzz band is covered by the
    # host-side fp64 straddle margin)
    wx = np.zeros((B, 64, 4), dtype=np.float32)
    for bg in range(B):
        xmn = np.float64(xl32[bg].min())
        xmx = np.float64(xl32[bg].max())
        wx[bg, :, 0] = -np.minimum(w * xmn, w * xmx)
    # device layout: [64, BL*4] per core, cols 4b:4b+4
    wxp = wx.reshape(NCORES, BL, 64, 4).transpose(0, 2, 1, 3).reshape(
        NCORES, 64, BL * 4)

    xlr = np.ascontiguousarray(
        xl32.reshape(NCORES, BL, 128, NJ).transpose(0, 2, 1, 3)
    ).reshape(NCORES, 128, BL * NJ)

    nc_m = _get_program("M7", _build_merged)
    in_maps = [
        {
            "xq": Xq[i],
            "w1big": w1big,
            "biasx": biasx,
            "cst128": cst128,
            "cst64": cst64,
            "ones1": np.ones((128, 128), dtype=np.float32),
            "wx": np.ascontiguousarray(wxp[i]),
            "xl": xlr[i],
        }
        for i in core_ids
    ]
    res = run_bass_kernel_spmd(nc_m, in_maps, core_ids)
    if res.exec_time_ns is not None:
        LAST_EXEC_NS.append(res.exec_time_ns)

    y = np.ascontiguousarray(
        np.concatenate(
            [np.asarray(res.results[i]["y"], dtype=np.float32)
             for i in core_ids],
            axis=0,
        ).transpose(0, 2, 1)
    )

    # ---- host verification of hinge classification (fp64, exact) ----
    mw2_ = np.asarray(mw2, dtype=np.float64)
    mb2_ = np.asarray(mb2, dtype=np.float64)
    iw1_ = np.asarray(iw1, dtype=np.float64)
    ib1_ = np.asarray(ib1, dtype=np.float64)
    iw2_ = np.asarray(iw2, dtype=np.float64)
    ib2_ = np.asarray(ib2, dtype=np.float64)
    for i in core_ids:
        hacc = np.asarray(res.results[i]["hacc"], dtype=np.float64)
        maskd = np.asarray(res.results[i]["mask"], dtype=np.float64)
        for bl in range(BL):
            bg = BL * i + bl
            hsum128 = hacc[bl].sum(axis=1) + n_dve_elems * b1cat
            hsum = hsum128[:H] + hsum128[H:]
            msg = mw2_.T @ hsum + N * mb2_
            c = iw1_[:M].T @ msg + ib1_
            xmn = np.float64(xl32[bg].min())
            xmx = np.float64(xl32[bg].max())
            lo = np.minimum(w * xmn, w * xmx) + c
            hi = np.maximum(w * xmn, w * xmx) + c
            on_dev = maskd[bl, :, 0] > 0.5
            xb = xl32[bg].astype(np.float64)
            # margin covering device fp32 chain error
            marg = 1e-4 * (np.abs(c) + np.abs(w) * max(abs(xmn), abs(xmx)) + 1e-9)
            straddle = (lo < marg) & (hi > -marg)
            wrong = (~straddle) & (on_dev != (lo > 0))
            fix = np.nonzero(straddle | wrong)[0]
            if len(fix):
                for hh in fix:
                    zh = w[hh] * xb + c[hh]
                    corr = np.maximum(zh, 0.0) - (1.0 if on_dev[hh] else 0.0) * zh
                    y[bg] += (iw2_[hh][None, :] * corr[:, None]).astype(np.float32)
    return y


# revision 13
# speedup vs baseline: 1.0392x; 1.0392x over previous
"""Trainium2 Bass kernel for nn_CustomModel_88862873354402 (gnn_message_passing).

Model (per batch b of 32, N=65536 nodes, D=16 features):
    h        = relu(X @ mw1 + mb1)               [N, 64]
    messages = h @ mw2 + mb2                     [N, 32]
    msg_sum  = sum_n messages                    [32]      (broadcast to all nodes)
    feat     = [msg_sum, x_last]                 [N, 33]
    g        = relu(feat @ iw1 + ib1)            [N, 64]
    out      = g @ iw2 + ib2                     [N, 3]

Algebraic structure exploited:
 1. msg_sum needs only sum_n relu(X @ mw1 + mb1), never per-node messages.
 2. Stage 2 collapses to an exact per-batch affine map out = A_b*x_last + B_b
    because |c_h| >> |w_h*x|; straddling hinges (classified on device, verified
    host-side in fp64 with a safety margin) are patched on host (n=0 here).

v7 design (117 us -> target ~100 us):
 - Single merged launch; X packed host-side into bf16 feature-major quadrant
   layout; four 32-row quadrant matmuls per 2048-col round at distinct
   tile_position row groups (bf16: 1 cyc/col).
 - relu+sum drains IN-PLACE on PSUM (out == in tile): ACT takes psum bank
   pair 0 (Relu bias=mb1, accum_out), DVE bank pair 1 (max(z,-b) + accum);
   in-place avoids the SBUF-output access penalty on ACT (222 vs 172 cyc)
   and drops the trash tiles.
 - Affine apply is EXACT (the model's output is affine in x_last given the
   hinge mask): y_o = A_o*x + B_o via tensor_scalar(mult,add) on the
   otherwise-idle GpSimd(Pool) engine - off the ACT/DVE critical path; the
   v6 Relu sign-trick and host sign-restore are gone. Last batch splits
   planes DVE/Pool/Pool so the tail applies run in parallel.
 - Per-batch coefficient chain on device as in v6 (generator interleaved
   into the NEXT batch's rounds; plain ops on GpSimd, reductions/max on
   DVE, 2 tiny PE matmuls with all-ones stationary for cross-partition
   reduce+broadcast). Sign-trick steps removed (3 fewer serial ops).
 - _prune_drain_deps now also prunes the PE queue's waits (in-order
   retirement makes repeated same-producer waits redundant).
 - DMA order: w1big + biasx + first x chunk first; other consts (needed
   only by the chain, ~25us in) issued after, so round 0 starts sooner.
"""
import sys

if "/opt/trn_rl_repo" not in sys.path:
    sys.path.insert(0, "/opt/trn_rl_repo")

from contextlib import ExitStack

import ml_dtypes
import numpy as np

import bass_rust as _bass_rust
import concourse.bass as bass
import concourse.tile as tile
from concourse import mybir
from concourse.bass_utils import run_bass_kernel_spmd

F32 = mybir.dt.float32
BF16 = mybir.dt.bfloat16
AF = mybir.ActivationFunctionType
ALU = mybir.AluOpType
BFNP = ml_dtypes.bfloat16

B, N, D = 32, 65536, 16
H, M, OUT = 64, 32, 3
NCORES = 8
BL = B // NCORES            # batches per core
CHUNK = 16384               # nodes per chunk tile
NCH = N // CHUNK            # chunks per batch
QCOLS = 2048                # moving cols per quadrant per chunk (2 nodes/col)
RND = 4                     # rounds per chunk (512 cols per quadrant each)
NRND = NCH * RND            # rounds per batch
NJ = N // 128               # apply free dim per batch
HCOLS = QCOLS // 2          # per-round cols drained by each of ACT / DVE

LAST_EXEC_NS = []

_cache = {}


def _finalize(nc):
    # Legalize for walrus: at most one sync wait per instruction.
    _bass_rust.move_matmul_waits_to_ldweights(nc.m)
    _bass_rust.generate_event_semaphores(nc)


_COMPUTE_ENGINES = ("EngineType.PE", "EngineType.Activation", "EngineType.DVE",
                    "EngineType.Pool", "EngineType.GpSimd", "EngineType.SP")


def _prune_drain_deps(nc):
    """Reduce sync deps using in-order engine-queue guarantees.

    Each compute engine retires its instructions in FIFO order, so:
    - several sync-deps on the same producer engine collapse to the latest;
    - a dep on instruction T is droppable if an earlier instruction on the
      SAME consumer engine already kept a sync-dep on T (any target kind,
      including a specific DMA instruction);
    - same-engine deps are implicit.
    Fewer deps -> fewer InstEventSemaphore instructions on engine queues.
    """
    f = list(nc.m.functions)[0]
    for blk in f.blocks:
        insts = list(blk.instructions)
        by_name = {i.name: i for i in insts}
        order = {i.name: k for k, i in enumerate(insts)}
        kept = set()  # (consumer_engine, dep_target) pairs already waited on
        for i in insts:
            eng = str(i.engine)
            if eng not in _COMPUTE_ENGINES:
                continue
            deps = [d for d, info in i.dependency_edges()]
            by_prod = {}
            for d in deps:
                if d not in by_name:
                    continue
                peng = str(by_name[d].engine)
                if peng == eng:
                    i.remove_dependency(d)
                elif (eng, d) in kept:
                    i.remove_dependency(d)
                elif peng in _COMPUTE_ENGINES and peng != "EngineType.SP":
                    by_prod.setdefault(peng, []).append(d)
                else:
                    kept.add((eng, d))
            for peng, ds in by_prod.items():
                ds.sort(key=lambda d: order[d])
                for d in ds[:-1]:
                    i.remove_dependency(d)
                kept.add((eng, ds[-1]))


def _build_merged():
    """Single launch: stage-1 relu-sum rounds + on-device per-batch affine
    coefficients + exact affine apply + y writeback. Host verifies hinge
    classification afterwards (fp64) and patches y for straddling hinges."""
    nc = bass.Bass()
    xq_in = nc.declare_dram_parameter("xq", [BL, NCH, 128, QCOLS], BF16, isOutput=False)
    w1_in = nc.declare_dram_parameter("w1big", [128, 128], BF16, isOutput=False)
    b1_in = nc.declare_dram_parameter("biasx", [128, 2], F32, isOutput=False)
    c128_in = nc.declare_dram_parameter("cst128", [128, 71], F32, isOutput=False)
    c64_in = nc.declare_dram_parameter("cst64", [64, 40], F32, isOutput=False)
    wx_in = nc.declare_dram_parameter("wx", [64, BL * 4], F32, isOutput=False)
    on_in = nc.declare_dram_parameter("ones1", [128, 128], F32, isOutput=False)
    xl_in = nc.declare_dram_parameter("xl", [128, BL * NJ], F32, isOutput=False)
    hacc_out = nc.declare_dram_parameter(
        "hacc", [BL, 128, 2 * NRND], F32, isOutput=True
    )
    mask_out = nc.declare_dram_parameter("mask", [BL, 64, 1], F32, isOutput=True)
    y_out = nc.declare_dram_parameter("y", [BL, OUT, N], F32, isOutput=True)

    with tile.TileContext(nc) as tc, ExitStack() as ctx:
        const_pool = ctx.enter_context(tc.tile_pool(name="const", bufs=1))
        xin_pool = ctx.enter_context(tc.tile_pool(name="xin", bufs=4))
        xb_pool = ctx.enter_context(tc.tile_pool(name="xb", bufs=2))
        acc_pool = ctx.enter_context(tc.tile_pool(name="acc", bufs=4))
        ch_pool = ctx.enter_context(tc.tile_pool(name="chain", bufs=2))
        ypool = ctx.enter_context(tc.tile_pool(name="yb", bufs=2))
        psA_pool = ctx.enter_context(tc.tile_pool(name="psA", bufs=2, space="PSUM"))
        psV_pool = ctx.enter_context(tc.tile_pool(name="psV", bufs=2, space="PSUM"))

        # group 1: needed by round 0 (stationary weights + drain bias).
        # Issued on the ACT HWDGE queue so they don't delay the first x
        # chunk on the SP queue.
        w1big = const_pool.tile([128, 128], BF16)
        nc.scalar.dma_start(out=w1big[:], in_=w1_in[:, :])
        biasx = const_pool.tile([128, 2], F32)
        nc.scalar.dma_start(out=biasx[:], in_=b1_in[:, :])
        bias = biasx[:, 0:1]
        negb = biasx[:, 1:2]
        # group 2 tiles (DMAs deferred until after the first x chunk so the
        # first round's data wins the queue)
        cst128 = const_pool.tile([128, 71], F32)
        cst64 = const_pool.tile([64, 40], F32)
        wxall = const_pool.tile([64, BL * 4], F32)
        ones_s = const_pool.tile([128, 128], F32)

        def emit_group2_dmas():
            nc.sync.dma_start(out=cst128[:], in_=c128_in[:, :])
            nc.sync.dma_start(out=cst64[:], in_=c64_in[:, :])
            nc.sync.dma_start(out=wxall[:], in_=wx_in[:, :])
            nc.sync.dma_start(out=ones_s[:], in_=on_in[:, :])

        w2big_s = cst128[:, 0:32]     # w2big[h or h+64, m] = mw2[h, m]
        ib2rep = cst128[:, 64:70]     # cols 3:6 = ib2, cols 0:3 = 0
        bcorr32 = cst128[:, 70:71]    # n_dve_elems * [mb1;mb1] / (2*NRND)
        iw1T_s = cst64[:, 0:32]       # iw1T[h, m] = iw1[m, h]
        iw2_s = cst64[:, 32:35]
        iw2w_s = cst64[:, 35:38]      # iw2[h, o] * w[h]
        ib1c32 = cst64[:, 38:39]      # ib1' / 32

        def chain_gen(b, acc, xb_t, last=False):
            # ge: engine for the plain add/mult chain ops. GpSimd for
            # interleaved batches; DVE for the LAST batch, where serial
            # latency (not queue occupancy) matters. Reductions / is_gt /
            # PSUM reads must stay on DVE (Pool rejects them).
            ge = nc.vector if last else nc.gpsimd
            wxb = wxall[:, 4 * b : 4 * b + 4]
            # hsc = sum over all 2*NRND acc cols (+ folded n_dve*mb1 corr)
            hsc = ch_pool.tile([128, 1], F32, tag="hsc")
            tr1 = ch_pool.tile([128, 2 * NRND], F32, tag="tr1")
            nc.vector.tensor_scalar(tr1[:], acc[:], bcorr32, None, op0=ALU.add,
                                    op1=ALU.add, accum_out=hsc[:])
            yield
            t32 = ch_pool.tile([128, 32], F32, tag="t32")
            nc.vector.tensor_scalar(t32[:], w2big_s, hsc[:], None, op0=ALU.mult)
            yield
            msg_ps = psA_pool.tile([64, 32], F32, tag="psa")
            nc.tensor.matmul(msg_ps[:], ones_s[:, 0:64], t32[:],
                             start=True, stop=True)
            yield
            # cc = iw1[:M].T @ msg + ib1' (ib1' folded into the reduce;
            # it carries ib1 + the N*mb2 contribution, host-side)
            tcm = ch_pool.tile([64, 32], F32, tag="tcm")
            if last:
                nc.vector.tensor_tensor(tcm[:], iw1T_s, msg_ps[:], op=ALU.mult)
            else:
                msgc = ch_pool.tile([64, 32], F32, tag="msgc")
                nc.vector.tensor_copy(msgc[:], msg_ps[:])
                yield
                ge.tensor_tensor(tcm[:], iw1T_s, msgc[:], op=ALU.mult)
            cc = ch_pool.tile([64, 1], F32, tag="cc")
            tc0 = ch_pool.tile([64, 32], F32, tag="tc0")
            nc.vector.tensor_scalar(tc0[:], tcm[:], ib1c32, None, op0=ALU.add,
                                    op1=ALU.add, accum_out=cc[:])
            yield
            # hinge mask via host-precomputed threshold tau (band errors are
            # re-verified and patched host-side in fp64)
            on = ch_pool.tile([64, 1], F32, tag="on")
            nc.vector.tensor_tensor(on[:], cc[:], wxb[:, 0:1], op=ALU.is_gt)
            # mask DMA issued here: its 64 tiny descriptors grind slowly on
            # the DMA engine, so start it early to hide under the chain tail
            nc.sync.dma_start(out=mask_out[b, :, :], in_=on[:])
            yield
            occ = ch_pool.tile([64, 1], F32, tag="occ")
            ge.tensor_tensor(occ[:], cc[:], on[:], op=ALU.mult)
            t6 = ch_pool.tile([64, 6], F32, tag="t6")
            nc.vector.tensor_scalar(t6[:, 0:3], iw2w_s, on[:], None,
                                    op0=ALU.mult)
            nc.vector.tensor_scalar(t6[:, 3:6], iw2_s, occ[:], None,
                                    op0=ALU.mult)
            yield
            scb_ps = psV_pool.tile([128, 6], F32, tag="psv")
            nc.tensor.matmul(scb_ps[:], ones_s[0:64, :], t6[:],
                             start=True, stop=True)
            yield
            sc2 = ch_pool.tile([128, 6], F32, tag="sc2")
            if last:
                nc.vector.tensor_tensor(sc2[:], scb_ps[:], ib2rep, op=ALU.add)
            else:
                scb = ch_pool.tile([128, 6], F32, tag="scb")
                nc.vector.tensor_copy(scb[:], scb_ps[:])
                yield
                ge.tensor_tensor(sc2[:], scb[:], ib2rep, op=ALU.add)
            yield
            # exact affine apply: y_o = A_o*x + B_o (no Relu needed - the
            # hinge-collapsed model IS affine). Off the ACT/DVE round path:
            # interleaved batches apply on GpSimd; last batch fans planes
            # across DVE/GpSimd so they run in parallel.
            yb = ypool.tile([128, OUT, NJ], F32)
            yv = y_out[b, :, :].rearrange("o (p j) -> p o j", p=128)
            # last batch: fan the plane DMAs across the ACT + SP HWDGE
            # queues (ACT idle by then) for parallel writeback; mid-stream
            # batches push y via the GpSimd software DGE so queue 1 stays
            # dedicated to the x input stream.
            dma_eng = [nc.scalar, nc.sync, nc.scalar] if last else [nc.gpsimd] * 3
            for o in range(OUT):
                eng = nc.vector if (last and o == 0) else nc.gpsimd
                eng.tensor_scalar(
                    yb[:, o, :], xb_t[:],
                    sc2[:, o : o + 1], sc2[:, 3 + o : 4 + o],
                    op0=ALU.mult, op1=ALU.add,
                )
                dma_eng[o].dma_start(out=yv[:, o : o + 1, :], in_=yb[:, o : o + 1, :])
                if o < OUT - 1 and not last:
                    yield

        pending = None
        for b in range(BL):
            acc = acc_pool.tile([128, 2 * NRND], F32, tag="acc")
            xb_t = xb_pool.tile([128, NJ], F32, tag="xb")
            for c in range(NCH):
                xt = xin_pool.tile([128, QCOLS], BF16)
                if b == 0 and c == 0:
                    # split first chunk so round 0 starts ~4x sooner
                    for r4 in range(RND):
                        nc.sync.dma_start(
                            out=xt[:, 512 * r4 : 512 * (r4 + 1)],
                            in_=xq_in[b, c, :, 512 * r4 : 512 * (r4 + 1)],
                        )
                else:
                    nc.sync.dma_start(out=xt[:], in_=xq_in[b, c, :, :])
                    if b == 0 and c == 1:
                        # consts for the chain (first needed ~25us in),
                        # issued after chunk 1 so they don't delay it
                        emit_group2_dmas()
                for r in range(RND):
                    psa = psA_pool.tile([128, HCOLS], F32, tag="psa")
                    psv = psV_pool.tile([128, HCOLS], F32, tag="psv")
                    for q in range(4):
                        ps = psa if q < 2 else psv
                        nc.tensor.matmul(
                            ps[:, 512 * (q % 2) : 512 * (q % 2 + 1)],
                            w1big[32 * q : 32 * (q + 1), :],
                            xt[32 * q : 32 * (q + 1), 512 * r : 512 * (r + 1)],
                            start=True,
                            stop=True,
                            tile_position=(32 * q, 0),
                        )
                    col = c * RND + r
                    # in-place drains: out == in psum tile (no SBUF trash)
                    nc.scalar.activation(
                        psa[:], psa[:], AF.Relu, bias=bias, scale=1.0,
                        accum_out=acc[:, col : col + 1],
                    )
                    nc.vector.tensor_scalar(
                        psv[:], psv[:], negb, None, op0=ALU.max, op1=ALU.add,
                        accum_out=acc[:, NRND + col : NRND + col + 1],
                    )
                    if pending is not None:
                        next(pending, None)
            nc.gpsimd.dma_start(out=xb_t[:], in_=xl_in[:, b * NJ : (b + 1) * NJ])
            # last batch's hacc goes out on the (now idle) ACT queue so its
            # 128 small descriptors don't serialize behind the y writeback
            hacc_eng = nc.scalar if b == BL - 1 else nc.sync
            hacc_eng.dma_start(out=hacc_out[b, :, :], in_=acc[:])
            if pending is not None:
                for _ in pending:
                    pass
            pending = chain_gen(b, acc, xb_t, last=(b == BL - 1))
        for _ in pending:
            pass
    _prune_drain_deps(nc)
    _finalize(nc)
    return nc


def _get_program(key, builder, *args):
    if key not in _cache:
        _cache[key] = builder(*args)
    return _cache[key]


def _pack_x(inputs):
    X = np.ascontiguousarray(np.asarray(inputs, dtype=np.float32))
    Xb = X.astype(BFNP)
    Xq = np.ascontiguousarray(
        Xb.reshape(NCORES, BL, NCH, 4, QCOLS, 2, D).transpose(0, 1, 2, 3, 5, 6, 4)
    ).reshape(NCORES, BL, NCH, 128, QCOLS)
    return X, Xq


def _w1big_biasx(mw1, mb1):
    w1big = np.zeros((128, 128), dtype=np.float32)
    for q in range(4):
        for e in range(2):
            w1big[32 * q + 16 * e : 32 * q + 16 * e + 16,
                  64 * e : 64 * e + 64] = mw1
    w1big = w1big.astype(BFNP)
    biasx = np.zeros((128, 2), dtype=np.float32)
    biasx[:, 0] = np.concatenate([mb1, mb1])
    biasx[:, 1] = -biasx[:, 0]
    return w1big, biasx


def kernel(inputs, mw1, mb1, mw2, mb2, iw1, ib1, iw2, ib2):
    global LAST_EXEC_NS
    LAST_EXEC_NS = []
    X, Xq = _pack_x(inputs)
    mw1 = np.asarray(mw1, dtype=np.float32)
    mb1 = np.asarray(mb1, dtype=np.float32)
    mw2f = np.asarray(mw2, dtype=np.float32)
    mb2f = np.asarray(mb2, dtype=np.float32)
    iw1f = np.asarray(iw1, dtype=np.float32)
    ib1f = np.asarray(ib1, dtype=np.float32)
    iw2f = np.asarray(iw2, dtype=np.float32)
    ib2f = np.asarray(ib2, dtype=np.float32)
    core_ids = list(range(NCORES))
    w1big, biasx = _w1big_biasx(mw1, mb1)
    b1cat = biasx[:, 0].astype(np.float64)
    n_dve_elems = NRND * HCOLS

    xl32 = X[:, :, D - 1]                        # [B, N] fp32
    w = iw1f[M, :].astype(np.float64)            # hinge slopes

    cst128 = np.zeros((128, 71), dtype=np.float32)
    cst128[0:H, 0:32] = mw2f
    cst128[H:128, 0:32] = mw2f
    cst128[:, 32:64] = (np.float64(N) * mb2f.astype(np.float64))[None, :]
    for o in range(OUT):
        cst128[:, 67 + o] = ib2f[o]
    cst128[:, 70] = (n_dve_elems * b1cat / (2 * NRND)).astype(np.float32)
    cst64 = np.zeros((64, 40), dtype=np.float32)
    cst64[:, 0:32] = iw1f[:M].T
    cst64[:, 32:35] = iw2f
    cst64[:, 35:38] = iw2f * iw1f[M, :][:, None]
    ib1p = (
        ib1f.astype(np.float64)
        + iw1f[:M].astype(np.float64).T @ (np.float64(N) * mb2f.astype(np.float64))
    )
    cst64[:, 38] = (ib1p / 32.0).astype(np.float32)

    # per-batch hinge threshold: on = cc > tau,
    # tau = -min(w*xmn, w*xmx) (the 1e-5 fuzz band is covered by the
    # host-side fp64 straddle margin)
    wx = np.zeros((B, 64, 4), dtype=np.float32)
    for bg in range(B):
        xmn = np.float64(xl32[bg].min())
        xmx = np.float64(xl32[bg].max())
        wx[bg, :, 0] = -np.minimum(w * xmn, w * xmx)
    # device layout: [64, BL*4] per core, cols 4b:4b+4
    wxp = wx.reshape(NCORES, BL, 64, 4).transpose(0, 2, 1, 3).reshape(
        NCORES, 64, BL * 4)

    xlr = np.ascontiguousarray(
        xl32.reshape(NCORES, BL, 128, NJ).transpose(0, 2, 1, 3)
    ).reshape(NCORES, 128, BL * NJ)

    nc_m = _get_program("M7", _build_merged)
    in_maps = [
        {
            "xq": Xq[i],
            "w1big": w1big,
            "biasx": biasx,
            "cst128": cst128,
            "cst64": cst64,
            "ones1": np.ones((128, 128), dtype=np.float32),
            "wx": np.ascontiguousarray(wxp[i]),
            "xl": xlr[i],
        }
        for i in core_ids
    ]
    res = run_bass_kernel_spmd(nc_m, in_maps, core_ids)
    if res.exec_time_ns is not None:
        LAST_EXEC_NS.append(res.exec_time_ns)

    y = np.ascontiguousarray(
        np.concatenate(
            [np.asarray(res.results[i]["y"], dtype=np.float32)
             for i in core_ids],
            axis=0,
        ).transpose(0, 2, 1)
    )

    # ---- host verification of hinge classification (fp64, exact) ----
    mw2_ = np.asarray(mw2, dtype=np.float64)
    mb2_ = np.asarray(mb2, dtype=np.float64)
    iw1_ = np.asarray(iw1, dtype=np.float64)
    ib1_ = np.asarray(ib1, dtype=np.float64)
    iw2_ = np.asarray(iw2, dtype=np.float64)
    ib2_ = np.asarray(ib2, dtype=np.float64)
    for i in core_ids:
        hacc = np.asarray(res.results[i]["hacc"], dtype=np.float64)
        maskd = np.asarray(res.results[i]["mask"], dtype=np.float64)
        for bl in range(BL):
            bg = BL * i + bl
            hsum128 = hacc[bl].sum(axis=1) + n_dve_elems * b1cat
            hsum = hsum128[:H] + hsum128[H:]
            msg = mw2_.T @ hsum + N * mb2_
            c = iw1_[:M].T @ msg + ib1_
            xmn = np.float64(xl32[bg].min())
            xmx = np.float64(xl32[bg].max())
            lo = np.minimum(w * xmn, w * xmx) + c
            hi = np.maximum(w * xmn, w * xmx) + c
            on_dev = maskd[bl, :, 0] > 0.5
            xb = xl32[bg].astype(np.float64)
            # margin covering device fp32 chain error
            marg = 1e-4 * (np.abs(c) + np.abs(w) * max(abs(xmn), abs(xmx)) + 1e-9)
            straddle = (lo < marg) & (hi > -marg)
            wrong = (~straddle) & (on_dev != (lo > 0))
            fix = np.nonzero(straddle | wrong)[0]
            if len(fix):
                for hh in fix:
                    zh = w[hh] * xb + c[hh]
                    corr = np.maximum(zh, 0.0) - (1.0 if on_dev[hh] else 0.0) * zh
                    y[bg] += (iw2_[hh][None, :] * corr[:, None]).astype(np.float32)
    return y
